# revision 1
# baseline (speedup 1.0000x reference)
"""Cross-attention (B=4, C=256, H=W=64) Trainium2 Bass kernel.

Math (per batch b), with t = target[b] : [C, N], r = reference[b], N = H*W:
    q = Wq t + bq ; k = Wk r + bk ; v = Wv r + bv
    attn = softmax(q^T k / sqrt(C), axis=j)
    out = v attn^T + t

Sharding: 8 cores = 4 batches x 2 query-halves. Each core handles its
query slice (NQ = 2048) against the full key/value set of its batch.

Algebraic folds (all exact):
  * scores: q_i . k_j = t_i^T (Wq^T Wk) r_j + bq.(Wk r_j) + (Wq t_i).bk + bq.bk
    The last two terms are per-query constants -> cancel in softmax.
    So with M = Wq^T Wk and g = Wk^T bq:  s[i,j] = r_j . u_i  where
    u = M^T t + g.
  * bv: softmax rows sum to 1, so v -> v + bv just adds bv to the output;
    the host adds it.
  * normalization: the device returns o[c,i] = sum_j v[c,j] exp(s_ij)
    plus a 128-way partial sum of E over keys (dacc); the host finishes
    the denominator colsum(E), divides, and adds the residual.

Work split: the 1x1-conv projections (u = M^T t + g, v = Wv r) are tiny
(~0.5% of the FLOPs) and run on the host in f32, quantized to the same
fp8 the device math consumes.  The device runs the attention core -- the
only O(N^2 C) work -- as a single software-pipelined loop:

    scores S^T[j_blk, i] = (r8 stationary) x (u8 moving), fp8 DoubleRow
    E = exp(S * scale + bias) on the ACT engine, fp32 PSUM -> fp8 SBUF
    out += (v8 stationary) x (E moving), fp8 DoubleRow, PSUM accumulate
    dacc[j_lo, i] += E[j_lo, (j_hi, i)] on the (otherwise idle) DVE

The exp stream is the critical path: 64 ACTIVATEs of [128,1024] at
(1024+352)/1.2GHz ~ 1.15us each ~ 73.4us; tensor work is ~62us and is
interleaved between score pairs so ACT never starves.  Startup: inputs
land in need-order chunks via both HW-DGE queues (SP + ACT) while
throwaway matmuls warm the PE clock gate and a dummy exp pulls in the
ACT table.  Each icp's last AV pair is deferred past the next icp's
first scores so the exp stream never pauses at the boundary.

Device layouts (matmuls contract over the partition axis):
    u8q[k]    : [128, 1024] fp8  [c_lo, (c_hi, i_loc)]   scores rhs
    r8_sb[ch] : [128, 2048] fp8 x4 [c_lo, (jb, c_hi, j_lo)] scores stationary
    v8_sb[ch] : [128, 2048] fp8 x4 [j_lo, (jb, c)]          AV stationary
    scores    : S^T[j_blk, (ic2, i)] in a [128, 1024] PSUM tile; one exp
               (ACT) per key block covering a PAIR of query chunks; the
               AV pass runs one key pair behind so exp latency hides.
"""

import os
import sys

import numpy as np

try:
    import concourse.bass as _probe  # noqa: F401
except ImportError:
    for _p in ("/opt/trn_rl_repo", "/root/.axon_site/_ro/trn_rl_repo"):
        if os.path.isdir(_p) and _p not in sys.path:
            sys.path.insert(0, _p)

import ml_dtypes

import concourse.bacc as bacc
import concourse.mybir as mybir
import concourse.tile as tile
from concourse.bass_utils import run_bass_kernel_spmd

BF16 = mybir.dt.bfloat16
FP8 = mybir.dt.float8e4
F32 = mybir.dt.float32
NPBF16 = ml_dtypes.bfloat16
NPFP8 = ml_dtypes.float8_e4m3

B, C, H, W = 4, 256, 64, 64
N = H * W                 # 4096 key/value pixels per batch
NCORES = 8
NQ = (B * N) // NCORES    # 2048 query pixels per core
P = 128
CB = C // P               # 2 channel blocks
ICH = 512                 # query chunk (one PSUM bank of fp32)
NICH = NQ // ICH          # 4
NJB = N // P              # 32 key blocks
SCALE = float(C) ** -0.5
EXP_BIAS = float(np.log(1 / 32.0))  # fp8e4m3 headroom (max finite 240, seen
                                    # scores reach ~7.9); the factor cancels
                                    # exactly in the numerator/denominator

# Set by test harness: trace=True to collect an NTFF profile.
TRACE = False
LAST_RESULTS = None


def _build():
    nc = bacc.Bacc("TRN2", target_bir_lowering=False, debug=False,
                   num_devices=NCORES)

    u8 = nc.dram_tensor("u8", [P, 2 * NQ], FP8, kind="ExternalInput")
    r8 = nc.dram_tensor("r8", [P, 2 * N], FP8, kind="ExternalInput")
    v8 = nc.dram_tensor("v8", [P, NJB * C], FP8, kind="ExternalInput")
    o = nc.dram_tensor("o", [C, NQ], F32, kind="ExternalOutput")
    dacc = nc.dram_tensor("dacc", [P, 2 * NQ], F32, kind="ExternalOutput")

    with tile.TileContext(nc) as tc:
        with (
            tc.tile_pool(name="persist", bufs=1) as persist,
            tc.tile_pool(name="epool", bufs=4) as epool,
            tc.tile_pool(name="outp", bufs=4) as outp,
            tc.tile_pool(name="ps_s", bufs=2, space="PSUM") as ps_s,
            tc.tile_pool(name="ps_av", bufs=4, space="PSUM") as ps_av,
        ):
            # ---- PE warmup first: ~3.4us of throwaway matmuls open the
            # HAM clock gate (1.2 -> 2.4 GHz) while the input DMAs land.
            warm = persist.tile([P, 512], BF16, tag="warm")
            nc.vector.memset(warm[:], 0.0)
            wps = ps_s.tile([P, 2 * ICH], F32, tag="s", name="wps")
            for i in range(8):
                nc.tensor.matmul(wps[:, :512], lhsT=warm[:, :P], rhs=warm[:],
                                 start=True, stop=True)

            # ---- inputs, in need-order chunks split across the two
            # HW-DGE queues (SP + ACT; ACT's queue is idle until the
            # first exp).  One queue drains ~150 GB/s with ~2us latency,
            # so the first score pair starts ~3us after the preamble.
            u8q = [persist.tile([P, RCH := 2 * ICH], FP8, tag=f"u8_{k}",
                                name=f"u8_{k}")
                   for k in range(4)]
            r8_sb = [persist.tile([P, 2048], FP8, tag=f"r8_{ch}",
                                  name=f"r8_{ch}") for ch in range(4)]
            v8_sb = [persist.tile([P, 2048], FP8, tag=f"v8_{ch}",
                                  name=f"v8_{ch}") for ch in range(4)]
            u8_ap = u8[:, :].rearrange("p (h n) -> p h n", h=2)

            def load_u8q(k, eng):
                eng.dma_start(
                    out=u8q[k][:].rearrange("p (h n) -> p h n", h=2),
                    in_=u8_ap[:, :, k * ICH:(k + 1) * ICH])

            nc.scalar.dma_start(out=r8_sb[0][:], in_=r8[:, 0:2048])
            nc.scalar.dma_start(out=v8_sb[0][:], in_=v8[:, 0:2048])
            for k in range(4):
                load_u8q(k, nc.sync)

            exp_bias = persist.tile([P, 1], F32, tag="expbias")
            nc.vector.memset(exp_bias[:], EXP_BIAS)
            # Dummy exp: forces the ~2.7us ACT table load off the critical
            # path — after the two gating DMA triggers, before the rest.
            dummy = persist.tile([P, 1], F32, tag="dummy")
            nc.scalar.activation(dummy[:], exp_bias[:],
                                 mybir.ActivationFunctionType.Exp)

            nc.scalar.dma_start(out=r8_sb[1][:], in_=r8[:, 2048:4096])
            nc.sync.dma_start(out=r8_sb[2][:], in_=r8[:, 4096:6144])
            nc.scalar.dma_start(out=v8_sb[1][:], in_=v8[:, 2048:4096])
            nc.sync.dma_start(out=v8_sb[2][:], in_=v8[:, 4096:6144])
            nc.scalar.dma_start(out=r8_sb[3][:], in_=r8[:, 6144:8192])
            nc.sync.dma_start(out=v8_sb[3][:], in_=v8[:, 6144:8192])

            def r8_ap(jb):
                # [c_lo, c_hi, j_lo] stationary block for key block jb
                ch, off = jb // 8, (jb % 8) * 2 * P
                return r8_sb[ch][:, off:off + 2 * P].rearrange(
                    "p (h j) -> p h j", h=2)

            def v8_ap(jpair, cb):
                # [j_lo, j_hi, c-chunk] stationary block for (jpair, cb)
                ch, off = jpair // 4, (jpair % 4) * 2 * C
                return v8_sb[ch][:, off:off + 2 * C].rearrange(
                    "p (h c) -> p h c", h=2)[:, :, cb * P:(cb + 1) * P]

            # ---- attention ----------------------------------------------
            # icp indexes PAIRS of query chunks (2 x 512 queries); per key
            # block jb: 2 score matmuls -> one [128,1024] exp -> fp8 E
            # tile per key pair; the AV pass runs one key pair behind in
            # 2-matmul units so exp latency and PSUM hazards stay hidden.
            NJ2 = NJB // 2
            # denominator partial-sum tiles: [j_lo, (j_hi, i)] per
            # (icp, ic2); the ic2=0 half accumulates on DVE, ic2=1 on the
            # otherwise-idle Pool engine, one [128,1024] add per pair per
            # engine so neither becomes the pacer.
            acc = [[persist.tile([P, 2 * ICH], F32, tag=f"acc{icp}_{ic2}",
                                 name=f"acc{icp}_{ic2}")
                    for ic2 in range(2)]
                   for icp in range(NICH // 2)]
            acc_eng = [nc.vector, nc.gpsimd]
            prev_tail = []
            for icp in range(NICH // 2):
                av = [ps_av.tile([P, ICH], F32, tag="av", name=f"av{icp}_{k}")
                      for k in range(2 * CB)]  # index = cb * 2 + ic2
                ets = {}

                def emit_scores(jb, icp=icp, ets=ets):
                    jpair, jhi = jb // 2, jb % 2
                    sps = ps_s.tile([P, 2 * ICH], F32, tag="s", name="sps")
                    for ic2 in range(2):
                        nc.tensor.matmul(
                            sps[:, ic2 * ICH:(ic2 + 1) * ICH],
                            lhsT=r8_ap(jb),
                            rhs=u8q[2 * icp + ic2][:].rearrange(
                                "p (h n) -> p h n", h=2),
                            start=True, stop=True,
                            perf_mode=mybir.MatmulPerfMode.DoubleRow,
                        )
                    if jhi == 0:
                        ets[jpair] = epool.tile([P, 4 * ICH], FP8, tag="e",
                                                name="et")
                    et = ets[jpair]
                    nc.scalar.activation(et[:, jhi * 2 * ICH:
                                            (jhi + 1) * 2 * ICH], sps[:],
                                         mybir.ActivationFunctionType.Exp,
                                         scale=SCALE, bias=exp_bias[:])
                    if jhi == 1:
                        # denominator partials: acc[icp][ic2] += E slice
                        # [j_lo, j_hi, ic2-half] (fp8 -> f32)
                        et3 = et.rearrange("p (h x) -> p h x", h=2)
                        for ic2 in range(2):
                            a3 = acc[icp][ic2][:].rearrange(
                                "p (h n) -> p h n", h=2)
                            esl = et3[:, :, ic2 * ICH:(ic2 + 1) * ICH]
                            if jpair == 0:
                                acc_eng[ic2].tensor_copy(out=a3, in_=esl)
                            else:
                                acc_eng[ic2].tensor_add(a3, a3, esl)

                def av_half(jpair, cb, icp=icp, av=av, ets=ets):
                    # one stationary v block (jpair, cb), both query chunks
                    et = ets[jpair] if cb < CB - 1 else ets.pop(jpair)
                    et3 = et.rearrange("p (h x) -> p h x", h=2)
                    final = jpair == NJ2 - 1
                    for ic2 in range(2):
                        k = cb * 2 + ic2
                        nc.tensor.matmul(
                            av[k][:],
                            lhsT=v8_ap(jpair, cb),
                            rhs=et3[:, :, ic2 * ICH:(ic2 + 1) * ICH],
                            start=(jpair == 0), stop=final,
                            perf_mode=mybir.MatmulPerfMode.DoubleRow,
                        )
                        if final:
                            # evacuate PSUM right behind the last matmul,
                            # freeing the bank for the next icp.  Mid-
                            # stream (icp0) only DVE may do this (ACT is
                            # the critical path, Pool can't read PSUM);
                            # after the last exp (icp1) ACT helps.
                            isl = slice((2 * icp + ic2) * ICH,
                                        (2 * icp + ic2 + 1) * ICH)
                            ot = outp.tile([P, ICH], F32, tag="o", name="ot")
                            if icp == 1 and k % 2 == 1:
                                nc.scalar.copy(ot[:], av[k][:])
                            else:
                                nc.vector.tensor_copy(out=ot[:], in_=av[k][:])
                            eng = nc.scalar if icp == 1 and cb == 1 else nc.sync
                            eng.dma_start(
                                out=o[cb * P:(cb + 1) * P, isl], in_=ot[:])

                emit_scores(0)
                emit_scores(1)
                for fn in prev_tail:
                    fn()
                prev_tail = []
                for jpair in range(1, NJ2):
                    emit_scores(2 * jpair)
                    av_half(jpair - 1, 0)
                    emit_scores(2 * jpair + 1)
                    av_half(jpair - 1, 1)
                # defer this icp's last AV pair past the next icp's first
                # scores so the exp stream never pauses at the boundary;
                # ship the denominator partials at the same point.
                def ship_dacc(icp=icp):
                    for ic2 in range(2):
                        eng = nc.sync if icp == 0 else nc.scalar
                        blk = (2 * icp + ic2) * 2 * ICH
                        eng.dma_start(out=dacc[:, blk:blk + 2 * ICH],
                                      in_=acc[icp][ic2][:])

                prev_tail = [
                    lambda fn=av_half: fn(NJ2 - 1, 0),
                    lambda fn=av_half: fn(NJ2 - 1, 1),
                    ship_dacc,
                ]
            for fn in prev_tail:
                fn()

    nc.finalize()
    return nc


_NC_CACHE = None


def kernel(target, reference, Wq, bq, Wk, bk, Wv, bv):
    global _NC_CACHE, LAST_RESULTS
    target = np.asarray(target, np.float32)
    reference = np.asarray(reference, np.float32)
    Wq, Wk, Wv = (np.asarray(w, np.float32) for w in (Wq, Wk, Wv))
    bq, bk, bv = (np.asarray(b_, np.float32) for b_ in (bq, bk, bv))

    if _NC_CACHE is None:
        _NC_CACHE = _build()
    nc = _NC_CACHE

    t_full = target.reshape(B, C, N)
    r_full = reference.reshape(B, C, N)
    m_full = Wq.T @ Wk                           # scores fold: M = Wq^T Wk
    g_col = (Wk.T @ bq).reshape(C, 1)            # bq fold (bk cancels exactly)
    in_maps = []
    for cid in range(NCORES):
        b_, h_ = cid // 2, cid % 2
        # u = M^T t + g in f32 on the host; fp8 in the DoubleRow moving
        # layout [c_lo, (c_hi, i)]
        u = m_full.T @ t_full[b_][:, h_ * NQ:(h_ + 1) * NQ] + g_col
        u8 = u.reshape(CB, P, NQ).transpose(1, 0, 2).reshape(P, 2 * NQ)
        # r8: DoubleRow stationary layout [c_lo, (jb, c_hi, j_local)]
        r8 = (r_full[b_].reshape(CB, P, NJB, P)
              .transpose(1, 2, 0, 3).reshape(P, 2 * N))
        # v = Wv r in f32 on the host; fp8 AV stationary layout
        # [j_lo, (jb, c)]
        v = Wv @ r_full[b_]
        v8 = v.reshape(C, NJB, P).transpose(2, 1, 0).reshape(P, NJB * C)
        in_maps.append({
            "u8": np.ascontiguousarray(u8).astype(NPFP8),
            "r8": np.ascontiguousarray(r8).astype(NPFP8),
            "v8": np.ascontiguousarray(v8).astype(NPFP8),
        })

    res = run_bass_kernel_spmd(
        nc, in_maps, core_ids=list(range(NCORES)), trace=TRACE,
    )
    LAST_RESULTS = res

    out = np.empty((B, C, N), np.float32)
    for cid in range(NCORES):
        b_, h_ = cid // 2, cid % 2
        o = res.results[cid]["o"].astype(np.float64)
        # dacc blocks per (icp, ic2): [j_lo, (j_hi, i)]; finish the
        # denominator by summing the partials over j_lo and j_hi.
        den = (res.results[cid]["dacc"].astype(np.float64)
               .reshape(P, NICH, 2, ICH).sum(axis=(0, 2)).reshape(NQ))
        sl = slice(h_ * NQ, (h_ + 1) * NQ)
        out[b_][:, sl] = (o / den[None, :] + bv.astype(np.float64)[:, None]
                          + t_full[b_][:, sl])
    return out.reshape(B, C, H, W)



# revision 2
# speedup vs baseline: 1.3221x; 1.3221x over previous
"""Cross-attention (B=4, C=256, H=W=64) Trainium2 Bass kernel.

Math (per batch b), with t = target[b] : [C, N], r = reference[b], N = H*W:
    q = Wq t + bq ; k = Wk r + bk ; v = Wv r + bv
    attn = softmax(q^T k / sqrt(C), axis=j)
    out = v attn^T + t

Sharding: 8 cores = 4 batches x 2 query-halves. Each core handles its
query slice (NQ = 2048) against the full key/value set of its batch.

Algebraic folds (all exact):
  * scores: q_i . k_j = t_i^T (Wq^T Wk) r_j + bq.(Wk r_j) + (Wq t_i).bk + bq.bk
    The last two terms are per-query constants -> cancel in softmax.
    So with M = Wq^T Wk and g = Wk^T bq:  s[i,j] = r_j . u_i  where
    u = M^T t + g.
  * bv: softmax rows sum to 1, so v -> v + bv just adds bv to the output;
    the host adds it.
  * normalization: the device returns o[c,i] = sum_j v[c,j] exp(s_ij)
    and den[i] = sum_j exp(s_ij); the host divides and adds the residual.

The 1x1-conv projections (u = M^T t + g, v = Wv r) are tiny (~0.5% of
the FLOPs) and run on the host in f32, quantized to the fp8 the device
math consumes.  The device runs the attention core -- the only O(N^2 C)
work -- as a single software-pipelined loop over 4 query chunks of 512
and 16 key-block PAIRS (256 keys each) per chunk:

    unit(jp): scores S^T[j, (jb2, q)] = 2 fp8 DoubleRow matmuls -> PSUM
              E = exp(S*scale+bias) : one [128,1024] ACTIVATE -> fp8 SBUF
              AV:  av[cb] += (v8 stationary [j,jb2,c]) x E, 2 matmuls
              den: dps    += (ones [j,jb2,16])         x E, 1 matmul

The ACT exp stream is the pacer: 64 ACTIVATEs x ~1.11us ~ 71us; the PE
runs 5 matmuls/unit (~1.08us pipelined) hidden under it.  The key-pair
DoubleRow AV (contracting j_lo x jb-pair) halves AV PSUM to 2 banks,
freeing one bank for the ones-matmul denominator: all 16 of its output
rows equal sum_j E[j, q], so no cross-partition reduction remains and
the old DVE/Pool accumulate streams (the previous pacer) vanish.

PSUM budget: scores 2 bufs x 2 banks + AV 3 bufs x 1 bank + den 1 = 8.

Device layouts (matmuls contract over partition x DoubleRow-pair):
    u8 : [c_lo, (chunk, c_hi, q)]    scores moving, contiguous per chunk
    r8 : [c_lo, (jb, c_hi, j_lo)]    scores stationary
    v8 : [j_lo, (jp, jb2, c)]        AV stationary
    o  : [c, i] f32 numerator; den : [1, i] f32 denominator.
"""

import os
import sys

import numpy as np

try:
    import concourse.bass as _probe  # noqa: F401
except ImportError:
    for _p in ("/opt/trn_rl_repo", "/root/.axon_site/_ro/trn_rl_repo"):
        if os.path.isdir(_p) and _p not in sys.path:
            sys.path.insert(0, _p)

import ml_dtypes

import concourse.bacc as bacc
import concourse.mybir as mybir
import concourse.tile as tile
from concourse.bass_utils import run_bass_kernel_spmd

BF16 = mybir.dt.bfloat16
FP8 = mybir.dt.float8e4
F32 = mybir.dt.float32
NPBF16 = ml_dtypes.bfloat16
NPFP8 = ml_dtypes.float8_e4m3

B, C, H, W = 4, 256, 64, 64
N = H * W                 # 4096 key/value pixels per batch
NCORES = 8
NQ = (B * N) // NCORES    # 2048 query pixels per core
P = 128
CB = C // P               # 2 channel blocks
ICH = 512                 # query chunk
NCH = NQ // ICH           # 4 chunks
NJB = N // P              # 32 key blocks
NJ2 = NJB // 2            # 16 key-block pairs
SCALE = float(C) ** -0.5
EXP_BIAS = float(np.log(1 / 32.0))  # fp8e4m3 headroom; cancels in the
                                    # numerator/denominator ratio

# Set by test harness: trace=True to collect an NTFF profile.
TRACE = False
LAST_RESULTS = None


def _build():
    nc = bacc.Bacc("TRN2", target_bir_lowering=False, debug=False,
                   num_devices=NCORES)

    u8 = nc.dram_tensor("u8", [P, NCH * 2 * ICH], FP8, kind="ExternalInput")
    r8 = nc.dram_tensor("r8", [P, 2 * N], FP8, kind="ExternalInput")
    v8 = nc.dram_tensor("v8", [P, NJ2 * 2 * C], FP8, kind="ExternalInput")
    o = nc.dram_tensor("o", [C, NQ], F32, kind="ExternalOutput")
    den = nc.dram_tensor("den", [1, NQ], F32, kind="ExternalOutput")

    with tile.TileContext(nc) as tc:
        with (
            tc.tile_pool(name="persist", bufs=1) as persist,
            tc.tile_pool(name="epool", bufs=4) as epool,
            tc.tile_pool(name="outp", bufs=4) as outp,
            tc.tile_pool(name="dsb", bufs=2) as dsb,
            tc.tile_pool(name="ps_s", bufs=2, space="PSUM") as ps_s,
            tc.tile_pool(name="ps_av", bufs=3, space="PSUM") as ps_av,
            tc.tile_pool(name="ps_d", bufs=1, space="PSUM") as ps_d,
        ):
            # ---- PE warmup first: throwaway matmuls open the HAM clock
            # gate (0.65 -> 2.4 GHz) while the input DMAs land.
            warm = persist.tile([P, 512], BF16, tag="warm")
            nc.vector.memset(warm[:], 0.0)
            wps = ps_s.tile([P, 2 * ICH], F32, tag="s", name="wps")
            for i in range(8):
                nc.tensor.matmul(wps[:, :512], lhsT=warm[:, :P], rhs=warm[:],
                                 start=True, stop=True)

            # ---- inputs, in need-order chunks split across the two
            # HW-DGE queues (SP + ACT; ACT's queue is idle until the
            # first exp).
            u8q = [persist.tile([P, 2 * ICH], FP8, tag=f"u8_{k}",
                                name=f"u8_{k}") for k in range(NCH)]
            r8_sb = [persist.tile([P, 2048], FP8, tag=f"r8_{ch}",
                                  name=f"r8_{ch}") for ch in range(4)]
            v8_sb = [persist.tile([P, 2048], FP8, tag=f"v8_{ch}",
                                  name=f"v8_{ch}") for ch in range(4)]

            nc.scalar.dma_start(out=r8_sb[0][:], in_=r8[:, 0:2048])
            for k in range(NCH):
                nc.sync.dma_start(out=u8q[k][:],
                                  in_=u8[:, k * 1024:(k + 1) * 1024])
            nc.scalar.dma_start(out=v8_sb[0][:], in_=v8[:, 0:2048])

            exp_bias = persist.tile([P, 1], F32, tag="expbias")
            nc.vector.memset(exp_bias[:], EXP_BIAS)
            ones = persist.tile([P, 32], FP8, tag="ones")
            nc.vector.memset(ones[:], 1.0)
            ones3 = ones[:].rearrange("p (h x) -> p h x", h=2)
            # Dummy exp: forces the ~1.3us ACT table load off the critical
            # path — after the two gating DMA triggers, before the rest.
            dummy = persist.tile([P, 1], F32, tag="dummy")
            nc.scalar.activation(dummy[:], exp_bias[:],
                                 mybir.ActivationFunctionType.Exp)

            nc.scalar.dma_start(out=r8_sb[1][:], in_=r8[:, 2048:4096])
            nc.sync.dma_start(out=r8_sb[2][:], in_=r8[:, 4096:6144])
            nc.scalar.dma_start(out=v8_sb[1][:], in_=v8[:, 2048:4096])
            nc.sync.dma_start(out=v8_sb[2][:], in_=v8[:, 4096:6144])
            nc.scalar.dma_start(out=r8_sb[3][:], in_=r8[:, 6144:8192])
            nc.sync.dma_start(out=v8_sb[3][:], in_=v8[:, 6144:8192])

            def r8_ap(jb):
                # [c_lo, c_hi, j_lo] stationary block for key block jb
                ch, off = jb // 8, (jb % 8) * 2 * P
                return r8_sb[ch][:, off:off + 2 * P].rearrange(
                    "p (h j) -> p h j", h=2)

            def v8_ap(jp, cb):
                # [j_lo, jb2, c-chunk] stationary block for (jp, cb)
                ch, off = jp // 4, (jp % 4) * 2 * C
                return v8_sb[ch][:, off:off + 2 * C].rearrange(
                    "p (h c) -> p h c", h=2)[:, :, cb * P:(cb + 1) * P]

            # ---- attention ----------------------------------------------
            prev_tail = []
            for k in range(NCH):
                av = [ps_av.tile([P, ICH], F32, tag="av",
                                 name=f"av{k}_{cb}") for cb in range(CB)]
                dp = ps_d.tile([16, ICH], F32, tag="dp", name=f"dp{k}")
                ets = {}

                def emit_scores(jp, k=k, ets=ets):
                    sps = ps_s.tile([P, 2 * ICH], F32, tag="s", name="sps")
                    for jbh in range(2):
                        nc.tensor.matmul(
                            sps[:, jbh * ICH:(jbh + 1) * ICH],
                            lhsT=r8_ap(2 * jp + jbh),
                            rhs=u8q[k][:].rearrange("p (h n) -> p h n", h=2),
                            start=True, stop=True,
                            perf_mode=mybir.MatmulPerfMode.DoubleRow,
                        )
                    et = epool.tile([P, 2 * ICH], FP8, tag="e", name="et")
                    ets[jp] = et
                    nc.scalar.activation(et[:], sps[:],
                                         mybir.ActivationFunctionType.Exp,
                                         scale=SCALE, bias=exp_bias[:])

                def av_den(jp, k=k, av=av, dp=dp, ets=ets):
                    et3 = ets.pop(jp).rearrange("p (h x) -> p h x", h=2)
                    final = jp == NJ2 - 1
                    for cb in range(CB):
                        nc.tensor.matmul(
                            av[cb][:], lhsT=v8_ap(jp, cb), rhs=et3,
                            start=(jp == 0), stop=final,
                            perf_mode=mybir.MatmulPerfMode.DoubleRow,
                        )
                    nc.tensor.matmul(
                        dp[:], lhsT=ones3, rhs=et3,
                        start=(jp == 0), stop=final,
                        perf_mode=mybir.MatmulPerfMode.DoubleRow,
                    )
                    if final:
                        # evacuate PSUM right behind the last matmuls.  On
                        # the last chunk ACT is done with exps and helps;
                        # mid-stream only DVE touches PSUM (ACT paces).
                        isl = slice(k * ICH, (k + 1) * ICH)
                        last = k == NCH - 1
                        for cb in range(CB):
                            ot = outp.tile([P, ICH], F32, tag="o", name="ot")
                            if last and cb == 1:
                                nc.scalar.copy(ot[:], av[cb][:])
                            else:
                                nc.vector.tensor_copy(out=ot[:], in_=av[cb][:])
                            nc.sync.dma_start(
                                out=o[cb * P:(cb + 1) * P, isl], in_=ot[:])
                        dt = dsb.tile([1, ICH], F32, tag="dt", name="dt")
                        nc.vector.tensor_copy(out=dt[:], in_=dp[0:1, :])
                        nc.sync.dma_start(out=den[0:1, isl], in_=dt[:])

                emit_scores(0)
                emit_scores(1)
                for fn in prev_tail:
                    fn()
                prev_tail = []
                for jp in range(2, NJ2):
                    emit_scores(jp)
                    av_den(jp - 2)
                av_den(NJ2 - 2)
                # defer this chunk's last AV/den unit past the next chunk's
                # first scores so the exp stream never pauses.
                prev_tail = [lambda fn=av_den: fn(NJ2 - 1)]
            for fn in prev_tail:
                fn()

    nc.finalize()
    return nc


_NC_CACHE = None


def kernel(target, reference, Wq, bq, Wk, bk, Wv, bv):
    global _NC_CACHE, LAST_RESULTS
    target = np.asarray(target, np.float32)
    reference = np.asarray(reference, np.float32)
    Wq, Wk, Wv = (np.asarray(w, np.float32) for w in (Wq, Wk, Wv))
    bq, bk, bv = (np.asarray(b_, np.float32) for b_ in (bq, bk, bv))

    if _NC_CACHE is None:
        _NC_CACHE = _build()
    nc = _NC_CACHE

    t_full = target.reshape(B, C, N)
    r_full = reference.reshape(B, C, N)
    m_full = Wq.T @ Wk                           # scores fold: M = Wq^T Wk
    g_col = (Wk.T @ bq).reshape(C, 1)            # bq fold (bk cancels exactly)
    in_maps = []
    for cid in range(NCORES):
        b_, h_ = cid // 2, cid % 2
        # u = M^T t + g in f32 on the host; fp8 [c_lo, (chunk, c_hi, q)]
        u = m_full.T @ t_full[b_][:, h_ * NQ:(h_ + 1) * NQ] + g_col
        u8 = (u.reshape(CB, P, NCH, ICH).transpose(1, 2, 0, 3)
              .reshape(P, NCH * 2 * ICH))
        # r8: stationary layout [c_lo, (jb, c_hi, j_lo)]
        r8 = (r_full[b_].reshape(CB, P, NJB, P)
              .transpose(1, 2, 0, 3).reshape(P, 2 * N))
        # v = Wv r in f32 on the host; fp8 AV stationary [j_lo, (jp, jb2, c)]
        v = Wv @ r_full[b_]
        v8 = (v.reshape(C, NJ2, 2, P).transpose(3, 1, 2, 0)
              .reshape(P, NJ2 * 2 * C))
        in_maps.append({
            "u8": np.ascontiguousarray(u8).astype(NPFP8),
            "r8": np.ascontiguousarray(r8).astype(NPFP8),
            "v8": np.ascontiguousarray(v8).astype(NPFP8),
        })

    res = run_bass_kernel_spmd(
        nc, in_maps, core_ids=list(range(NCORES)), trace=TRACE,
    )
    LAST_RESULTS = res

    out = np.empty((B, C, N), np.float32)
    for cid in range(NCORES):
        b_, h_ = cid // 2, cid % 2
        o = res.results[cid]["o"].astype(np.float64)
        d = res.results[cid]["den"].astype(np.float64).reshape(NQ)
        sl = slice(h_ * NQ, (h_ + 1) * NQ)
        out[b_][:, sl] = (o / d[None, :] + bv.astype(np.float64)[:, None]
                          + t_full[b_][:, sl])
    return out.reshape(B, C, H, W)
